# revision 3
# baseline (speedup 1.0000x reference)
"""Per-sample dynamic conv2d (VALID) on 8 Trainium2 NeuronCores.

Problem: X [32,128,128,128] f32 (NHWC), kernel [32,3,3,128,128] f32 (per-sample
HWIO) -> out [32,126,126,128] f32.

Sharding: pure data-parallel over batch; each of the 8 cores runs 4 samples.

Per-core algorithm (per sample b), "flipped-stationary" form:
  1. X rows are HWDGE-loaded [w, (h c)] f32, cast to bf16 (DVE), then
     transposed to channel-major XT [ci, (h w)] by the DMA XBAR
     (dma_start_transpose, 14ns per 16x128 tile) - no PE involvement.
  2. Kernel taps kt [ci, (kh kw) co] bf16 via gpsimd casting DMA.
  3. Matmuls: for each output chunk of CH=4 rows, accumulate all 9 taps in
     one PSUM bank:  pacc[co, (h w128)] += kt_tap[ci,co].T @ XT[ci, a:a+512]
     with a=(h0+kh)*128+kw.  The kernel tap is the *stationary* operand so
     consecutive chunks stream 512-row moving operands with the weight
     reload fully pipelined; a shifted conv window is just an offset flat
     slice of XT (w'=126,127 are overrun garbage, never stored).
  4. Drain: DVE copies pacc -> ob bf16; PE transposes (lagged one chunk so
     the PE never waits on the drain) ob -> ptp [w, (h co)] in PSUM; ACT
     copies ptp -> ot f32; HWDGE stores 4 NHWC rows per chunk.

Rationale: PE issue rate is gated by LDWEIGHTS+sem dispatch, not clock
(measured 822ns per 3x384-col row group vs 480ns of streaming at 2.4GHz).
This layout runs 9 matmuls of 512 moving rows per 13 PE instructions.
"""

import numpy as np

import concourse.bass as bass
import concourse.mybir as mybir
from concourse.bass_utils import run_bass_kernel_spmd
from concourse.masks import make_identity
from concourse.tile import TileContext

N_CORES = 8
B, H, W, C = 32, 128, 128, 128
KK = 3
BL = B // N_CORES            # samples per core
HO = WO = H - KK + 1         # 126
GR = 16                      # input rows per load/cast/xbar group
NG = H // GR                 # 8 groups per sample
CH = 4                       # output rows per PSUM chunk (4*128 <= 512)
XT_SZ = H * W + 128          # xt free size (pad: last chunk reads 2 past end)

F32 = mybir.dt.float32
BF16 = mybir.dt.bfloat16


def _split_excess_waits(nc, limit=1):
    """walrus codegen rejects >1 sync-wait on several instruction kinds.
    Move excess waits onto preceding same-engine NoOps."""
    n = 0
    for bb in nc.m.functions[0].blocks:
        out = []
        changed = False
        for inst in bb.instructions:
            si = inst.sync_info
            if si is not None and len(si.on_wait) > limit:
                waits = list(si.on_wait)
                excess, keep = waits[:-limit], waits[-limit:]
                for i in range(0, len(excess), limit):
                    n += 1
                    out.append(
                        mybir.InstNoOp(
                            name=f"I-waitsplit-{n}",
                            engine=inst.engine,
                            bass_nofuse=True,
                            sync_info=mybir.SyncInfo(
                                on_wait=excess[i : i + limit], on_update=[]
                            ),
                        )
                    )
                inst.sync_info = mybir.SyncInfo(on_wait=keep, on_update=si.on_update)
                changed = True
            out.append(inst)
        if changed:
            bb.instructions = out
    return n


def _build():
    nc = bass.Bass()
    Xd = nc.declare_dram_parameter("X", [BL, H, W, C], F32, isOutput=False)
    Kd = nc.declare_dram_parameter("kern", [BL, KK, KK, C, C], F32, isOutput=False)
    Od = nc.declare_dram_parameter("out", [BL, HO, WO, C], F32, isOutput=True)

    with TileContext(nc) as tc:
        with (
            tc.tile_pool(name="const", bufs=1) as p_const,
            tc.tile_pool(name="xs", bufs=3) as p_xs,
            tc.tile_pool(name="xb", bufs=3) as p_xb,
            tc.tile_pool(name="xt", bufs=2) as p_xt,
            tc.tile_pool(name="kt", bufs=2) as p_kt,
            tc.tile_pool(name="ob", bufs=4) as p_ob,
            tc.tile_pool(name="ot", bufs=4) as p_ot,
            tc.tile_pool(name="pacc", bufs=6, space="PSUM") as p_pacc,
            tc.tile_pool(name="ptp", bufs=2, space="PSUM") as p_ptp,
        ):
            identf = p_const.tile([128, 128], F32, tag="identf")
            make_identity(nc, identf[:, :])
            ident = p_const.tile([128, 128], BF16, tag="ident")
            nc.vector.tensor_copy(ident[:, :], identf[:, :])

            def emit_T(b):
                """Thunks emitting sample b's load/cast/xbar phase piecewise
                so it interleaves into the previous sample's matmul phase."""
                kt = p_kt.tile([C, KK * KK * C], BF16, tag="kt")
                nc.gpsimd.dma_start(
                    out=kt[:, :].rearrange("p (t co) -> p t co", t=KK * KK),
                    in_=Kd[b].rearrange("kh kw ci co -> ci (kh kw) co"),
                )
                xt = p_xt.tile([C, XT_SZ], BF16, tag="xt")
                state = {"kt": kt, "xt": xt}

                def thunks():
                    for g in range(NG):
                        def load(g=g):
                            xs = p_xs.tile([W, GR * C], F32, tag="xs")
                            nc.sync.dma_start(
                                out=xs[:, :].rearrange("w (h c) -> w h c", h=GR),
                                in_=Xd[b, g * GR : (g + 1) * GR].rearrange(
                                    "h w c -> w h c"
                                ),
                            )
                            state["xs"] = xs
                        yield load

                        def cast(g=g):
                            xb = p_xb.tile([W, GR * C], BF16, tag="xb")
                            nc.vector.tensor_copy(xb[:, :], state["xs"][:, :])
                            state["xb"] = xb
                        yield cast

                        def xbar(g=g):
                            nc.scalar.dma_start_transpose(
                                xt[:, g * GR * 128 : (g + 1) * GR * 128].rearrange(
                                    "c (h w) -> c h w", h=GR
                                ),
                                state["xb"][:, :],
                            )
                        yield xbar

                state["thunks"] = thunks()
                return state

            n_chunks = (HO + CH - 1) // CH  # 32 (31 full + 1 of 2 rows)

            def flush(b, ob, ch, h0):
                """Transpose+drain+store one finished chunk."""
                ptp = p_ptp.tile([128, 512], BF16, tag="ptp")
                for j in range(ch):
                    nc.tensor.transpose(
                        ptp[:, j * 128 : (j + 1) * 128],
                        ob[:, j * 128 : (j + 1) * 128],
                        ident[:, :],
                    )
                ot = p_ot.tile([128, 512], F32, tag="ot")
                nc.scalar.copy(ot[0:WO, : ch * 128], ptp[0:WO, : ch * 128])
                nc.sync.dma_start(
                    out=Od[b, h0 : h0 + ch].rearrange("h w c -> w h c"),
                    in_=ot[0:WO, : ch * 128].rearrange("w (h c) -> w h c", h=ch),
                )

            def emit_M(b, st, nxt):
                kt, xt = st["kt"], st["xt"]
                prev = None
                for ic in range(n_chunks):
                    h0 = CH * ic
                    ch = min(CH, HO - h0)
                    if nxt is not None:
                        t = next(nxt["thunks"], None)
                        if t is not None:
                            t()
                    pacc = p_pacc.tile([C, 512], F32, tag="pacc")
                    for t9 in range(KK * KK):
                        kh, kw = divmod(t9, KK)
                        a = (h0 + kh) * 128 + kw
                        nc.tensor.matmul(
                            pacc[:, : ch * 128],
                            kt[:, t9 * C : (t9 + 1) * C],
                            xt[:, a : a + ch * 128],
                            start=(t9 == 0),
                            stop=(t9 == KK * KK - 1),
                        )
                    ob = p_ob.tile([C, 512], BF16, tag="ob")
                    nc.vector.tensor_copy(ob[:, : ch * 128], pacc[:, : ch * 128])
                    if prev is not None:
                        flush(b, *prev)
                    prev = (ob, ch, h0)
                if nxt is not None:
                    for t in nxt["thunks"]:
                        t()
                flush(b, *prev)

            st = emit_T(0)
            for t in st["thunks"]:
                t()
            st["thunks"] = iter(())
            for b in range(BL):
                nxt = emit_T(b + 1) if b + 1 < BL else None
                emit_M(b, st, nxt)
                st = nxt

    _split_excess_waits(nc)
    return nc


_CACHE = {}


def _get_nc():
    if "nc" not in _CACHE:
        _CACHE["nc"] = _build()
    return _CACHE["nc"]


def _run(X, kern, **kw):
    in_maps = [
        {
            "X": np.ascontiguousarray(X[c * BL : (c + 1) * BL]),
            "kern": np.ascontiguousarray(kern[c * BL : (c + 1) * BL]),
        }
        for c in range(N_CORES)
    ]
    last_err = None
    for _attempt in range(3):
        try:
            res = run_bass_kernel_spmd(
                _get_nc(), in_maps, list(range(N_CORES)), **kw
            )
            break
        except Exception as e:  # transient NRT_EXEC_UNIT_UNRECOVERABLE etc.
            last_err = e
    else:
        raise last_err
    out = np.concatenate([res.results[c]["out"] for c in range(N_CORES)], axis=0)
    return out, res


def kernel(X, kernel):
    X = np.ascontiguousarray(X, dtype=np.float32)
    kern = np.ascontiguousarray(kernel, dtype=np.float32)
    out, _ = _run(X, kern)
    return out


# revision 8
# speedup vs baseline: 1.1347x; 1.1347x over previous
"""Per-sample dynamic conv2d (VALID) on 8 Trainium2 NeuronCores.

Problem: X [32,128,128,128] f32 (NHWC), kernel [32,3,3,128,128] f32 (per-sample
HWIO) -> out [32,126,126,128] f32.

Sharding: pure data-parallel over batch; each of the 8 cores runs 4 samples.

Per-core algorithm (per sample b), "flipped-stationary" form:
  1. X rows are HWDGE-loaded [w, (h c)] f32, cast to bf16 (DVE), then
     transposed to channel-major XT [ci, (h w)] by the DMA XBAR
     (dma_start_transpose, 14ns per 16x128 tile) - no PE involvement.
  2. Kernel taps kt [ci, (kh kw) co] bf16 via gpsimd casting DMA.
  3. Matmuls: for each output chunk of CH=4 rows, accumulate all 9 taps in
     one PSUM bank:  pacc[co, (h w128)] += kt_tap[ci,co].T @ XT[ci, a:a+512]
     with a=(h0+kh)*128+kw.  The kernel tap is the *stationary* operand so
     consecutive chunks stream 512-row moving operands with the weight
     reload fully pipelined; a shifted conv window is just an offset flat
     slice of XT (w'=126,127 are overrun garbage, never stored).
  4. Drain: DVE copies pacc -> ob bf16; PE transposes (lagged one chunk so
     the PE never waits on the drain) ob -> ptp [w, (h co)] in PSUM; ACT
     copies ptp -> ot f32; HWDGE stores 4 NHWC rows per chunk.

Rationale: PE issue rate is gated by LDWEIGHTS+sem dispatch, not clock
(measured 822ns per 3x384-col row group vs 480ns of streaming at 2.4GHz).
This layout runs 9 matmuls of 512 moving rows per 13 PE instructions.
"""

import numpy as np

import concourse.bass as bass
import concourse.mybir as mybir
from concourse.bass_utils import run_bass_kernel_spmd
from concourse.masks import make_identity
from concourse.tile import TileContext

N_CORES = 8
B, H, W, C = 32, 128, 128, 128
KK = 3
BL = B // N_CORES            # samples per core
HO = WO = H - KK + 1         # 126
GR = 16                      # input rows per load/cast/xbar group
NG = H // GR                 # 8 groups per sample
CH = 4                       # output rows per PSUM chunk (4*128 <= 512)
XT_SZ = H * W + 128          # xt free size (pad: last chunk reads 2 past end)

F32 = mybir.dt.float32
BF16 = mybir.dt.bfloat16


def _split_excess_waits(nc, limit=1):
    """walrus codegen rejects >1 sync-wait on several instruction kinds.
    Move excess waits onto preceding same-engine NoOps."""
    n = 0
    for bb in nc.m.functions[0].blocks:
        out = []
        changed = False
        for inst in bb.instructions:
            si = inst.sync_info
            if si is not None and len(si.on_wait) > limit:
                waits = list(si.on_wait)
                excess, keep = waits[:-limit], waits[-limit:]
                for i in range(0, len(excess), limit):
                    n += 1
                    out.append(
                        mybir.InstNoOp(
                            name=f"I-waitsplit-{n}",
                            engine=inst.engine,
                            bass_nofuse=True,
                            sync_info=mybir.SyncInfo(
                                on_wait=excess[i : i + limit], on_update=[]
                            ),
                        )
                    )
                inst.sync_info = mybir.SyncInfo(on_wait=keep, on_update=si.on_update)
                changed = True
            out.append(inst)
        if changed:
            bb.instructions = out
    return n


def _build():
    nc = bass.Bass()
    Xd = nc.declare_dram_parameter("X", [BL, H, W, C], F32, isOutput=False)
    Kd = nc.declare_dram_parameter("kern", [BL, KK, KK, C, C], F32, isOutput=False)
    Od = nc.declare_dram_parameter("out", [BL, HO, WO, C], F32, isOutput=True)

    with TileContext(nc) as tc:
        with (
            tc.tile_pool(name="const", bufs=1) as p_const,
            tc.tile_pool(name="xs", bufs=3) as p_xs,
            tc.tile_pool(name="xb", bufs=3) as p_xb,
            tc.tile_pool(name="xt", bufs=2) as p_xt,
            tc.tile_pool(name="kt", bufs=2) as p_kt,
            tc.tile_pool(name="ob", bufs=4) as p_ob,
            tc.tile_pool(name="ot", bufs=4) as p_ot,
            tc.tile_pool(name="pacc", bufs=5, space="PSUM") as p_pacc,
            tc.tile_pool(name="ptp", bufs=3, space="PSUM") as p_ptp,
        ):
            identf = p_const.tile([128, 128], F32, tag="identf")
            make_identity(nc, identf[:, :])
            ident = p_const.tile([128, 128], BF16, tag="ident")
            nc.vector.tensor_copy(ident[:, :], identf[:, :])

            def emit_T(b):
                """Thunks emitting sample b's load/cast/xbar phase piecewise
                so it interleaves into the previous sample's matmul phase."""
                kt = p_kt.tile([C, KK * KK * C], BF16, tag="kt")
                nc.gpsimd.dma_start(
                    out=kt[:, :].rearrange("p (t co) -> p t co", t=KK * KK),
                    in_=Kd[b].rearrange("kh kw ci co -> ci (kh kw) co"),
                )
                xt = p_xt.tile([C, XT_SZ], BF16, tag="xt")
                state = {"kt": kt, "xt": xt}

                if b == 0:
                    ranges = [(0, 2), (2, 4), (4, 8), (8, 16)] + [
                        (h0, h0 + GR) for h0 in range(GR, H, GR)
                    ]
                else:
                    ranges = [(h0, h0 + GR) for h0 in range(0, H, GR)]

                def thunks():
                    for h0, h1 in ranges:
                        n = h1 - h0

                        def load(h0=h0, n=n):
                            xs = p_xs.tile([W, GR * C], F32, tag="xs")
                            nc.sync.dma_start(
                                out=xs[:, : n * C].rearrange(
                                    "w (h c) -> w h c", h=n
                                ),
                                in_=Xd[b, h0 : h0 + n].rearrange("h w c -> w h c"),
                            )
                            state["xs"] = xs
                        yield load

                        def cast(n=n):
                            xb = p_xb.tile([W, GR * C], BF16, tag="xb")
                            nc.gpsimd.tensor_copy(
                                xb[:, : n * C], state["xs"][:, : n * C]
                            )
                            state["xb"] = xb
                        yield cast

                        def xbar(h0=h0, n=n):
                            nc.sync.dma_start_transpose(
                                xt[:, h0 * 128 : (h0 + n) * 128].rearrange(
                                    "c (h w) -> c h w", h=n
                                ),
                                state["xb"][:, : n * C],
                            )
                        yield xbar

                state["thunks"] = thunks()
                return state

            n_chunks = (HO + CH - 1) // CH  # 32 (31 full + 1 of 2 rows)

            def flush(b, ob, ch, h0):
                """Transpose+drain+store one finished chunk.  The PSUM->SBUF
                drain alternates DVE/ACT so neither queue gates ptp recycle."""
                ptp = p_ptp.tile([128, 512], BF16, tag="ptp")
                for j in range(ch):
                    nc.tensor.transpose(
                        ptp[:, j * 128 : (j + 1) * 128],
                        ob[:, j * 128 : (j + 1) * 128],
                        ident[:, :],
                    )
                ot = p_ot.tile([128, 512], F32, tag="ot")
                if (h0 // CH) % 2 == 0:
                    nc.scalar.copy(ot[0:WO, : ch * 128], ptp[0:WO, : ch * 128])
                else:
                    nc.vector.tensor_copy(ot[0:WO, : ch * 128], ptp[0:WO, : ch * 128])
                nc.scalar.dma_start(
                    out=Od[b, h0 : h0 + ch].rearrange("h w c -> w h c"),
                    in_=ot[0:WO, : ch * 128].rearrange("w (h c) -> w h c", h=ch),
                )

            def emit_M(b, st, nxt):
                kt, xt = st["kt"], st["xt"]
                prev = None
                for ic in range(n_chunks):
                    h0 = CH * ic
                    ch = min(CH, HO - h0)
                    if nxt is not None:
                        t = next(nxt["thunks"], None)
                        if t is not None:
                            t()
                    pacc = p_pacc.tile([C, 512], F32, tag="pacc")
                    for t9 in range(KK * KK):
                        kh, kw = divmod(t9, KK)
                        a = (h0 + kh) * 128 + kw
                        nc.tensor.matmul(
                            pacc[:, : ch * 128],
                            kt[:, t9 * C : (t9 + 1) * C],
                            xt[:, a : a + ch * 128],
                            start=(t9 == 0),
                            stop=(t9 == KK * KK - 1),
                        )
                    ob = p_ob.tile([C, 512], BF16, tag="ob")
                    nc.vector.tensor_copy(ob[:, : ch * 128], pacc[:, : ch * 128])
                    if prev is not None:
                        flush(b, *prev)
                    prev = (ob, ch, h0)
                if nxt is not None:
                    for t in nxt["thunks"]:
                        t()
                flush(b, *prev)

            st = emit_T(0)
            for t in st["thunks"]:
                t()
            st["thunks"] = iter(())
            for b in range(BL):
                nxt = emit_T(b + 1) if b + 1 < BL else None
                emit_M(b, st, nxt)
                st = nxt

    _split_excess_waits(nc)
    return nc


_CACHE = {}


def _get_nc():
    if "nc" not in _CACHE:
        _CACHE["nc"] = _build()
    return _CACHE["nc"]


def _run(X, kern, **kw):
    in_maps = [
        {
            "X": np.ascontiguousarray(X[c * BL : (c + 1) * BL]),
            "kern": np.ascontiguousarray(kern[c * BL : (c + 1) * BL]),
        }
        for c in range(N_CORES)
    ]
    last_err = None
    for _attempt in range(3):
        try:
            res = run_bass_kernel_spmd(
                _get_nc(), in_maps, list(range(N_CORES)), **kw
            )
            break
        except Exception as e:  # transient NRT_EXEC_UNIT_UNRECOVERABLE etc.
            last_err = e
    else:
        raise last_err
    out = np.concatenate([res.results[c]["out"] for c in range(N_CORES)], axis=0)
    return out, res


def kernel(X, kernel):
    X = np.ascontiguousarray(X, dtype=np.float32)
    kern = np.ascontiguousarray(kernel, dtype=np.float32)
    out, _ = _run(X, kern)
    return out
